# revision 15
# baseline (speedup 1.0000x reference)
"""Trainium2 Bass kernel for ChannelCompression:
   y = minmax_norm_spatial(leaky_relu(circulant_1x1_conv(x) + b))

Sharding: pure data parallel over batch (16 batches -> 2 per core x 8 cores).

Per-core schedule (memory-roofline bound: read x once, write y once):

  phase A:  batch-0 x loads as SWDGE cast-DMAs (f32 HBM -> bf16 SBUF) on
            the gpsimd ring, ramped op sizes so compute starts while the
            DMA clocks ramp.  Conv chunks: PE matmul (bf16 weights+ifmap,
            fp32 PSUM accumulate -- fp32 weights would serialize on
            weight reloads at ~3.4us/chunk) -> 2048-col PSUM -> ScalarE
            Prelu -> bf16 resident y.  DVE tensor_tensor min/max running
            accumulators (bf16 2x perf mode; tensor_reduce is 1x-capped),
            then fold stats into per-partition scale/bias.
  phase B:  batch-0 stores stream on the sync ring while batch-1
            cast-loads continue on the gpsimd ring interleaved with
            GpSimd normalizes; batch-1 Prelus own the ACT sequencer (a
            store's sem-wait there would head-block them).  Deep f32
            staging pool lets normalize run ahead of stores so y slots
            recycle at compute pace; batch-1 compute finishes early and
            its stats fold hides inside B.
  phase C:  batch-1 stores on the sync ring.

  - View each batch as [C=16, G=8, S=32768] and stack (c,g) onto the 128
    SBUF partitions.  The circulant 16x16 conv becomes one 128x128
    block-structured matmul weight kron(W2.T, I8), so every PE column
    computes all 16 output channels for 8 spatial groups at once.
  - Stats fold: per-partition [128,2] min/max -> PE transpose into free
    dim -> DVE group-reduce over the 8 spatial groups -> reciprocal ->
    broadcast back to [128,2] scale/bias with two tiny selector matmuls.
    Fold PSUM tiles share the main psum pool; PSUM->SBUF copies on DVE.
"""

import numpy as np
from contextlib import ExitStack

import concourse.bacc as bacc
import concourse.tile as tile
import concourse.bass as bass
from concourse import mybir
from concourse.bass_utils import run_bass_kernel_spmd

F32 = mybir.dt.float32
BF16 = mybir.dt.bfloat16
AF = mybir.ActivationFunctionType
ALU = mybir.AluOpType
AX = mybir.AxisListType

N_CORES = 8
B, C, H, W = 16, 16, 512, 512
G = 8                   # spatial groups stacked into partitions
BP = B // N_CORES       # batches per core
S_FULL = (H * W) // G   # 32768 spatial elems per group
CH = 2048               # column chunk: 1 PSUM tile, 1 ACT op, 1 y chunk
N_CH = S_FULL // CH     # chunks per batch (16)
MM = 512                # columns per matmul (1 PSUM bank output limit)
XT = 4096               # big load op columns (2 MiB HBM read)
EPS = 1e-8
NEG_SLOPE = 0.1


def build_nc():
    nc = bacc.Bacc("TRN2", target_bir_lowering=False)

    xs = nc.dram_tensor("x", [BP, C, G, S_FULL], F32, kind="ExternalInput")
    wbd = nc.dram_tensor("wbd", [128, 128], BF16, kind="ExternalInput")
    ident = nc.dram_tensor("ident", [128, 128], F32, kind="ExternalInput")
    sel = nc.dram_tensor("sel", [32, 2, 128], F32, kind="ExternalInput")
    bb = nc.dram_tensor("b128", [128, 1], F32, kind="ExternalInput")
    ys = nc.dram_tensor("y", [BP, C, G, S_FULL], F32, kind="ExternalOutput")

    with tile.TileContext(nc) as tc, ExitStack() as ctx:
        consts = ctx.enter_context(tc.tile_pool(name="consts", bufs=1))
        xpool0 = ctx.enter_context(tc.tile_pool(name="xpool0", bufs=4))
        xpool1 = ctx.enter_context(tc.tile_pool(name="xpool1", bufs=2))
        ypool = ctx.enter_context(tc.tile_pool(name="ypool", bufs=N_CH + 6))
        opool = ctx.enter_context(tc.tile_pool(name="opool", bufs=6))
        apool = ctx.enter_context(tc.tile_pool(name="acc", bufs=1))
        small = ctx.enter_context(tc.tile_pool(name="small", bufs=2))
        psum = ctx.enter_context(tc.tile_pool(name="psum", bufs=2, space="PSUM"))

        # consts on the scalar (ACT HWDGE) ring: idle early, and keeps the
        # load rings free
        wbd_sb = consts.tile([128, 128], BF16)
        nc.scalar.dma_start(out=wbd_sb, in_=wbd[:])
        id_sb = consts.tile([128, 128], F32)
        nc.scalar.dma_start(out=id_sb, in_=ident[:])
        sel_sb = consts.tile([32, 2, 128], F32)
        nc.scalar.dma_start(out=sel_sb, in_=sel[:])
        b_sb = consts.tile([128, 1], F32)
        nc.scalar.dma_start(out=b_sb, in_=bb[:])

        state = {}

        def conv_chunk(bi, j, xt, xoff):
            """Matmul+Prelu chunk j (CH cols) of batch bi from bf16 x tile
            xt (chunk starts at column xoff within xt); fold into min/max
            accs via bf16 tensor_tensor (2x DVE perf mode)."""
            acc_min, acc_max, y_chunks = state[bi]
            yt = ypool.tile([128, CH], BF16, tag="y")
            pt = psum.tile([128, CH], F32, tag="ps")
            for k in range(CH // MM):
                c0 = xoff + k * MM
                nc.tensor.matmul(
                    pt[:, k * MM:(k + 1) * MM],
                    wbd_sb,
                    xt[:, c0:c0 + MM],
                    start=True,
                    stop=True,
                )
            # y = leaky_relu(conv + b): fused PSUM->SBUF(bf16) on ScalarE
            nc.scalar.activation(
                out=yt, in_=pt, func=AF.Prelu, bias=b_sb, scale=1.0,
                alpha=NEG_SLOPE,
            )
            if j == 0:
                nc.vector.tensor_copy(acc_min, yt)
                nc.vector.tensor_copy(acc_max, yt)
            else:
                nc.vector.tensor_tensor(out=acc_min, in0=acc_min, in1=yt, op=ALU.min)
                nc.vector.tensor_tensor(out=acc_max, in0=acc_max, in1=yt, op=ALU.max)
            y_chunks.append(yt)

        def pass1_b0():
            """Batch-0 cast-loads on the gpsimd (SWDGE) ring, ramped: 2 x
            1 MiB-read then 2 MiB-read ops (first chunk lands fast)."""
            j = 0
            for _ in range(2):  # 1 MiB-read ops
                xt = xpool1.tile([128, CH], BF16, tag="x1")
                nc.gpsimd.dma_start(out=xt, in_=xs[0, :, :, j * CH:(j + 1) * CH])
                conv_chunk(0, j, xt, 0)
                j += 1
            while j < N_CH:  # 2 MiB-read ops
                xt = xpool0.tile([128, XT], BF16, tag="x0")
                c0 = j * CH
                nc.gpsimd.dma_start(out=xt, in_=xs[0, :, :, c0:c0 + XT])
                for c in range(XT // CH):
                    conv_chunk(0, j, xt, c * CH)
                    j += 1

        def pass1_b1_tile(t):
            """Batch-1: one 2 MiB-read cast-load on the gpsimd ring + 2
            conv chunks (Prelus own the ACT sequencer)."""
            xt = xpool0.tile([128, XT], BF16, tag="x0")
            nc.gpsimd.dma_start(out=xt, in_=xs[1, :, :, t * XT:(t + 1) * XT])
            for c in range(XT // CH):
                conv_chunk(1, t * (XT // CH) + c, xt, c * CH)

        def stats_fold(bi):
            """Fold accumulators into per-partition scale/bias [128,2].
            All PSUM->SBUF copies on DVE so the ACT sequencer stays free."""
            acc_min, acc_max = state[bi][:2]
            s2 = small.tile([128, 2], F32, tag="s2")
            nc.vector.tensor_reduce(out=s2[:, 0:1], in_=acc_min, axis=AX.X, op=ALU.min)
            nc.vector.tensor_reduce(out=s2[:, 1:2], in_=acc_max, axis=AX.X, op=ALU.max)
            # transpose [128,1] stats into free dim (partition 0)
            ptr_min = psum.tile([1, 128], F32, tag="ps")
            nc.tensor.transpose(ptr_min, s2[:, 0:1], id_sb)
            ptr_max = psum.tile([1, 128], F32, tag="ps")
            nc.tensor.transpose(ptr_max, s2[:, 1:2], id_sb)
            tl = small.tile([1, 256], F32, tag="tl")
            nc.vector.tensor_copy(tl[:, 0:128], ptr_min)
            nc.vector.tensor_copy(tl[:, 128:256], ptr_max)
            # reduce over the 8 groups (free index p = o*8+g)
            u = small.tile([1, 32], F32, tag="u")
            nc.vector.tensor_reduce(
                out=u[:, 0:16],
                in_=tl[:, 0:128].rearrange("p (o g) -> p o g", g=G),
                axis=AX.X,
                op=ALU.min,
            )
            nc.vector.tensor_reduce(
                out=u[:, 16:32],
                in_=tl[:, 128:256].rearrange("p (o g) -> p o g", g=G),
                axis=AX.X,
                op=ALU.max,
            )
            # scale = 1/(mx-mn+eps); nbias = -mn*scale
            vv = small.tile([1, 16], F32, tag="vv")
            nc.vector.scalar_tensor_tensor(
                out=vv, in0=u[:, 16:32], scalar=EPS, in1=u[:, 0:16],
                op0=ALU.add, op1=ALU.subtract,
            )
            pk = small.tile([1, 32], F32, tag="pk")
            nc.vector.reciprocal(out=pk[:, 0:16], in_=vv)
            nc.vector.scalar_tensor_tensor(
                out=pk[:, 16:32], in0=u[:, 0:16], scalar=-1.0, in1=pk[:, 0:16],
                op0=ALU.mult, op1=ALU.mult,
            )
            # broadcast [1,32] free-dim -> per-partition [128,2] via transpose
            # + selector matmuls (sel[k,0,p]=d(k==p//8), sel[k,1,p]=d(k-16==p//8))
            pz = psum.tile([32, 1], F32, tag="ps")
            nc.tensor.transpose(pz, pk, id_sb[0:1, 0:1])
            zs = small.tile([32, 1], F32, tag="zs")
            nc.vector.tensor_copy(zs, pz)
            pb1 = psum.tile([128, 1], F32, tag="ps")
            nc.tensor.matmul(pb1, sel_sb[:, 0, :], zs, start=True, stop=True)
            pb2 = psum.tile([128, 1], F32, tag="ps")
            nc.tensor.matmul(pb2, sel_sb[:, 1, :], zs, start=True, stop=True)
            sc = small.tile([128, 2], F32, tag="sc")
            nc.vector.tensor_copy(sc[:, 0:1], pb1)
            nc.vector.tensor_copy(sc[:, 1:2], pb2)
            return sc

        def pass2_chunk(bi, j, sc, norm_eng=None, store_eng=None):
            """Normalize resident bf16 y chunk into f32 staging and stream
            out.  Defaults: GpSimd normalize, sync-ring store (phase B);
            phase C alternates engines/rings since everything is idle."""
            y_chunks = state[bi][2]
            ot = opool.tile([128, CH], F32, tag="o")
            (norm_eng or nc.gpsimd).tensor_scalar(
                out=ot,
                in0=y_chunks[j],
                scalar1=sc[:, 0:1],
                scalar2=sc[:, 1:2],
                op0=ALU.mult,
                op1=ALU.add,
            )
            (store_eng or nc.sync).dma_start(
                out=ys[bi, :, :, j * CH:(j + 1) * CH], in_=ot
            )

        for bi in range(BP):
            state[bi] = (
                apool.tile([128, CH], BF16, tag=f"accmin{bi}", name=f"accmin{bi}"),
                apool.tile([128, CH], BF16, tag=f"accmax{bi}", name=f"accmax{bi}"),
                [],
            )
        pass1_b0()
        sc0 = stats_fold(0)
        for j in range(N_CH):
            pass2_chunk(0, j, sc0)
            if j % 2 == 0:
                pass1_b1_tile(j // 2)
        sc1 = stats_fold(1)
        for j in range(N_CH):
            pass2_chunk(
                1, j, sc1,
                norm_eng=nc.gpsimd if j % 2 == 0 else nc.vector,
                store_eng=nc.sync if j % 2 == 0 else nc.scalar,
            )

    nc.compile()
    return nc


def host_consts(w, b):
    """Host-side tiny constant tensors fed to every core."""
    w = np.asarray(w, np.float32).reshape(16)
    b = np.asarray(b, np.float32).reshape(1)
    W2 = np.stack([np.roll(w, o) for o in range(16)], axis=0)   # [O,C]
    wbd = np.kron(W2.T.copy(), np.eye(G, dtype=np.float32))     # [128,128]
    import ml_dtypes
    wbd_bf16 = np.ascontiguousarray(wbd).astype(ml_dtypes.bfloat16)
    ident = np.eye(128, dtype=np.float32)
    sel = np.zeros((32, 2, 128), np.float32)
    for p in range(128):
        sel[p // G, 0, p] = 1.0
        sel[16 + p // G, 1, p] = 1.0
    b128 = np.full((128, 1), float(b[0]), np.float32)
    return wbd_bf16, ident, sel, b128


_NC = None
LAST_RESULTS = None


def kernel(x, w, b):
    global _NC, LAST_RESULTS
    x = np.ascontiguousarray(np.asarray(x, np.float32))
    assert x.shape == (B, C, H, W)
    if _NC is None:
        _NC = build_nc()
    wbd_bf16, ident, sel, b128 = host_consts(w, b)

    xg = x.reshape(N_CORES, BP, C, G, S_FULL)
    in_maps = [
        {
            "x": np.ascontiguousarray(xg[ci]),
            "wbd": wbd_bf16,
            "ident": ident,
            "sel": sel,
            "b128": b128,
        }
        for ci in range(N_CORES)
    ]
    res = run_bass_kernel_spmd(_NC, in_maps, core_ids=list(range(N_CORES)))
    LAST_RESULTS = res
    out = np.concatenate([r["y"].reshape(BP, C, H, W) for r in res.results], axis=0)
    return out
